# revision 21
# baseline (speedup 1.0000x reference)
"""Trainium2 Bass kernel for nn_DrugEncoder2Real2Imag (3-layer GCN, 4 streams,
mean-pool per graph), graph-data-parallel across 8 NeuronCores.

Self-contained: hardcodes all shapes; takes FULL inputs, returns FULL outputs.

Math:  GCNConv out = D^-1/2 (A+I) D^-1/2 (X W) + b   (deg includes self-loop)
       => out = dinv * ((A+I) g) + b   with  g = dinv * (X W)
Self-loops are explicit edges, so aggregation is a plain segment-sum of
gathered g rows.  BN+bias+ReLU folds into a per-channel affine c1*z + c0.

Sharding: dst-node windows of 128; device d owns windows [98d, 98d+98).
Per-layer gather tables hold all 4 streams interleaved per node row
(512B/1KB/2KB rows -> full DMA bandwidth).  Edges are grouped per
(dst-window, src-chunk) — dma_gather uses int16 indices, so the node space
is split into 4 chunks of 25088 rows.  Aggregation = per-128-edge-tile
indicator matmul (is_equal(iota, dst_local)) accumulated in the window's
PSUM.  Layer boundaries: each device computes its G-table shard densely
(PE transposes + small matmuls), assembled via AllGather.  Pooling uses the
same indicator-matmul trick over graph slots in 4 persistent PSUM banks.
"""

LAST_RESULTS = None

import os

import numpy as np

# problem constants (the harness always calls with these shapes)
N = 100000
G = 2000
BN_EPS = 1e-5

P = 128
NDEV = 8
WPD = 98                      # windows per device
NW = NDEV * WPD               # 784 windows
NPAD = NW * P                 # 100352
NPD = WPD * P                 # 12544 nodes per device
GPAD = 512                    # pooled graph slots per device (4 PSUM banks)
NCHUNK = 4
CHROWS = NPAD // NCHUNK       # 25088 gather-table rows per chunk


# ---------------------------------------------------------------------------
def _legalize_waits(nc, mybir, max_waits=1):
    """This walrus build accepts only one semaphore wait per instruction;
    Tile attaches several.  Move excess waits onto injected same-engine nops."""
    n_split = 0
    for f in nc.m.functions:
        for bb in f.blocks:
            if not any(
                ins.sync_info is not None and len(ins.sync_info.on_wait) > max_waits
                for ins in bb.instructions
            ):
                continue
            new_list = []
            for ins in bb.instructions:
                si = ins.sync_info
                if si is not None and len(si.on_wait) > max_waits:
                    waits = list(si.on_wait)
                    si.on_wait = waits[:max_waits]
                    for w in waits[max_waits:]:
                        nop = mybir.InstNoOp(
                            name=nc.get_next_instruction_name(),
                            ins=[],
                            outs=[],
                            engine=ins.engine,
                            sync_info=mybir.SyncInfo(on_wait=[w], on_update=[]),
                        )
                        new_list.append(nop)
                        n_split += 1
                new_list.append(ins)
            bb.instructions = new_list
    return n_split


# ---------------------------------------------------------------------------
def _prep_host(x, edge_index, batch, params):
    """All graph preprocessing in numpy.  Returns per-core inputs + metadata."""
    src = np.asarray(edge_index[0], dtype=np.int64)
    dst = np.asarray(edge_index[1], dtype=np.int64)
    batch = np.asarray(batch, dtype=np.int64)

    deg = np.bincount(dst, minlength=N).astype(np.float64) + 1.0
    dinv = (1.0 / np.sqrt(deg)).astype(np.float32)
    dinv_pad = np.zeros(NPAD, np.float32)
    dinv_pad[:N] = dinv

    # edges + self loops, grouped by (dst window, src chunk)
    src_all = np.concatenate([src, np.arange(N, dtype=np.int64)])
    dst_all = np.concatenate([dst, np.arange(N, dtype=np.int64)])
    win = dst_all >> 7
    chunk = src_all // CHROWS
    key = win * NCHUNK + chunk
    order = np.argsort(key, kind="stable")
    src_s, dst_s = src_all[order], dst_all[order]
    gcnt = np.bincount(key[order], minlength=NW * NCHUNK)
    gstart = np.zeros(NW * NCHUNK + 1, np.int64)
    np.cumsum(gcnt, out=gstart[1:])

    # per (local window j, chunk c): padded count, uniform across devices
    cnt_djc = gcnt.reshape(NDEV, WPD, NCHUNK)
    mx_jc = cnt_djc.max(axis=0)
    m_jc = ((mx_jc + P - 1) // P) * P                      # [WPD, NCHUNK]
    nreal_jc = ((mx_jc + 15) // 16) * 16                   # gathered slots
    tiles_jc = m_jc // P

    # group order for paired gather calls: (pair, chunk, window-within-pair)
    NPAIR = WPD // 2
    group_seq = []                       # flat order of (j, c) groups
    for jp in range(NPAIR):
        for c in range(NCHUNK):
            for jw in range(2):
                group_seq.append((2 * jp + jw, c))
    gt_tiles = np.array([tiles_jc[j, c] for (j, c) in group_seq], np.int64)
    gt_off = np.zeros(len(group_seq) + 1, np.int64)
    np.cumsum(gt_tiles, out=gt_off[1:])
    T = int(gt_off[-1])
    # per-(j,c) tile offset lookup
    toff_jc = np.zeros((WPD, NCHUNK), np.int64)
    for gi, (j, c) in enumerate(group_seq):
        toff_jc[j, c] = gt_off[gi]

    idx16 = np.zeros((NDEV, 128, T * 8), np.int16)   # 8 int16 cols per tile
    dst_arr = np.full((NDEV, T * P), -1.0, np.float32)
    for d in range(NDEV):
        for j in range(WPD):
            w = d * WPD + j
            for c in range(NCHUNK):
                m = int(m_jc[j, c])
                if m == 0:
                    continue
                gidx = w * NCHUNK + c
                s0, e0 = gstart[gidx], gstart[gidx + 1]
                loc = np.zeros(m, np.int64)          # pad slots -> row 0
                loc[: e0 - s0] = src_s[s0:e0] - c * CHROWS
                t0 = int(toff_jc[j, c])
                blk = loc.reshape(m // 16, 16).T.astype(np.int16)
                for g8 in range(8):
                    idx16[d, g8 * 16 : g8 * 16 + 16, t0 * 8 : t0 * 8 + m // 16] = blk
                dl = np.full(m, -1.0, np.float32)
                dl[: e0 - s0] = (dst_s[s0:e0] - (w << 7)).astype(np.float32)
                dst_arr[d, t0 * P : t0 * P + m] = dl
    dst_arr = np.ascontiguousarray(dst_arr.reshape(NDEV, T, P).transpose(0, 2, 1))

    # pooling columns: 4 per node-window (graph slot = batch - base - 128q)
    batch_pad = np.full(NPAD, -(10 ** 6), np.int64)
    batch_pad[:N] = batch
    gbase = np.array([batch[min(d * NPD, N - 1)] for d in range(NDEV)], np.int64)
    for d in range(NDEV):
        hi = min((d + 1) * NPD, N)
        span = batch[hi - 1] - gbase[d] + 1
        assert span <= GPAD, f"device {d} spans {span} graphs > {GPAD}"
    pool_arr = np.zeros((NDEV, P, WPD, 4), np.float32)
    for d in range(NDEV):
        nodes = batch_pad[d * NPD : (d + 1) * NPD].reshape(WPD, P)  # [j, p]
        rel = (nodes - gbase[d]).astype(np.float64)
        for q in range(4):
            pool_arr[d, :, :, q] = (rel - 128 * q).T.astype(np.float32)

    # ---- parameters ----
    pr = {k: np.asarray(v, np.float64) for k, v in params.items() if k != "layers"}
    layers = [
        {k: (np.asarray(v, np.float64) if not isinstance(v, dict)
             else {kk: np.asarray(vv, np.float64) for kk, vv in v.items()})
         for k, v in lp.items()}
        for lp in params["layers"]
    ]

    # initial projection folded with layer-0 conv weights (both linear):
    # g1 = dinv * ((x @ Wp + bp) @ Wl0) = dinv * (x @ (Wp@Wl0) + bp@Wl0)
    weff = np.zeros((16, 128), np.float64)
    weff[:15, 0:64] = pr["Wp1"] @ layers[0]["W1"]
    weff[15, 0:64] = pr["bp1"] @ layers[0]["W1"]
    weff[:15, 64:128] = pr["Wp2"] @ layers[0]["W2"]
    weff[15, 64:128] = pr["bp2"] @ layers[0]["W2"]
    weff = weff.astype(np.float32)

    x_fm = np.zeros((16, NPAD), np.float32)
    x_fm[:15, :N] = np.asarray(x, np.float32).T
    x_fm[15, :] = 1.0

    # per-layer BN folds; stream order (r1, r2, i1, i2)
    def fold(lp, douts):
        c1 = np.zeros(4 * douts, np.float64)
        c0 = np.zeros(4 * douts, np.float64)
        for s_, (bkey, bnkey) in enumerate(
            [("b1", "bn_r1"), ("b2", "bn_r2"), ("b1", "bn_i1"), ("b2", "bn_i2")]
        ):
            bn = lp[bnkey]
            b = lp[bkey]
            sc = bn["gamma"] / np.sqrt(bn["var"] + BN_EPS)
            c1[s_ * douts : (s_ + 1) * douts] = sc
            c0[s_ * douts : (s_ + 1) * douts] = (b - bn["mean"]) * sc + bn["beta"]
        return c1.astype(np.float32), c0.astype(np.float32)

    c1_1, c0_1 = fold(layers[0], 64)
    c1_2, c0_2 = fold(layers[1], 64)
    c1_3, c0_3 = fold(layers[2], 128)

    wd2 = np.concatenate(
        [layers[1]["W1"], layers[1]["W2"], layers[1]["W1"], layers[1]["W2"]], axis=1
    ).astype(np.float32)  # [64, 256]
    wd3 = np.concatenate(
        [layers[2]["W1"], layers[2]["W2"], layers[2]["W1"], layers[2]["W2"]], axis=1
    ).astype(np.float32)  # [64, 512]
    # duplicated across both partition halves (matmul needs matching
    # base_partition for lhsT and rhs)
    wd2 = np.vstack([wd2, wd2])  # [128, 256]
    wd3 = np.vstack([wd3, wd3])  # [128, 512]

    iota = np.broadcast_to(np.arange(P, dtype=np.float32), (P, P)).copy()
    ident = np.eye(P, dtype=np.float32)
    dinv_w = np.ascontiguousarray(dinv_pad.reshape(NW, P).T)  # [128, 784]

    common = {
        "x_fm": x_fm,
        "weff": weff,
        "wd2": wd2,
        "wd3": wd3,
        "c1_1": np.broadcast_to(c1_1, (P, 256)).copy(),
        "c0_1": np.broadcast_to(c0_1, (P, 256)).copy(),
        "c1_2": np.broadcast_to(c1_2, (P, 256)).copy(),
        "c0_2": np.broadcast_to(c0_2, (P, 256)).copy(),
        "c1_3": np.broadcast_to(c1_3, (P, 512)).copy(),
        "c0_3": np.broadcast_to(c0_3, (P, 512)).copy(),
        "iota": iota,
        "iota_bf": iota.astype(np.dtype("bfloat16") if hasattr(np, "bfloat16") else np.float32) if False else iota,
        "ident": ident,
        "dinv_w": dinv_w,
    }
    in_maps = []
    for d in range(NDEV):
        m = dict(common)
        m["eidx"] = idx16[d]
        m["edst"] = dst_arr[d]
        m["dinv_loc"] = dinv_w[:, d * WPD : (d + 1) * WPD].copy()
        m["pool_cols"] = pool_arr[d].reshape(P, WPD * 4)
        in_maps.append(m)

    counts = np.maximum(np.bincount(batch, minlength=G).astype(np.float32), 1.0)
    meta = {"tiles_jc": tiles_jc, "m_jc": m_jc, "nreal_jc": nreal_jc,
            "toff_jc": toff_jc, "tpw": tiles_jc.sum(axis=1), "T": T,
            "gbase": gbase, "counts": counts}
    return in_maps, meta


# ---------------------------------------------------------------------------
def _build_program(meta):
    import concourse.bass as bass
    import concourse.bacc as bacc
    import concourse.mybir as mybir
    from concourse.tile import TileContext

    f32 = mybir.dt.float32
    bf16 = mybir.dt.bfloat16
    fp32_tables = os.environ.get("KERNEL_TABLE_FP32", "") == "1"
    tdt = f32 if fp32_tables else bf16
    tiles_jc = meta["tiles_jc"]
    m_jc = meta["m_jc"]
    nreal_jc = meta["nreal_jc"]
    toff_jc = meta["toff_jc"]
    T = meta["T"]
    NPAIR = WPD // 2

    nc = bacc.Bacc("TRN2", num_devices=NDEV)

    # ---- I/O ----
    x_fm = nc.dram_tensor("x_fm", [16, NPAD], f32, kind="ExternalInput")
    weff = nc.dram_tensor("weff", [16, 128], f32, kind="ExternalInput")
    wd2 = nc.dram_tensor("wd2", [128, 256], f32, kind="ExternalInput")
    wd3 = nc.dram_tensor("wd3", [128, 512], f32, kind="ExternalInput")
    cio = {}
    for nm, w in [("c1_1", 256), ("c0_1", 256), ("c1_2", 256), ("c0_2", 256),
                  ("c1_3", 512), ("c0_3", 512)]:
        cio[nm] = nc.dram_tensor(nm, [P, w], f32, kind="ExternalInput")
    iota_d = nc.dram_tensor("iota", [P, P], f32, kind="ExternalInput")
    ident_d = nc.dram_tensor("ident", [P, P], f32, kind="ExternalInput")
    dinv_w_d = nc.dram_tensor("dinv_w", [P, NW], f32, kind="ExternalInput")
    dinv_l_d = nc.dram_tensor("dinv_loc", [P, WPD], f32, kind="ExternalInput")
    eidx_d = nc.dram_tensor("eidx", [P, T * 8], mybir.dt.int16, kind="ExternalInput")
    edst_d = nc.dram_tensor("edst", [P, T], f32, kind="ExternalInput")
    pool_d = nc.dram_tensor("pool_cols", [P, WPD * 4], f32, kind="ExternalInput")
    pool_out = nc.dram_tensor("pool_out", [GPAD, 512], f32, kind="ExternalOutput")

    with TileContext(nc) as tc:
        with (
            tc.tile_pool(name="const", bufs=1) as cpool,
            tc.tile_pool(name="sbuf", bufs=3) as sbuf,
            tc.tile_pool(name="gpool", bufs=2) as gpool,
            tc.tile_pool(name="psum", bufs=2, space="PSUM") as psum,
            tc.tile_pool(name="pool_ps", bufs=1, space="PSUM") as ppool,
            tc.tile_pool(name="dram", bufs=1, space="DRAM") as dram,
        ):
            # ---- constants to SBUF ----
            def load_const(srcd, shape, dt=f32, name=None):
                t = cpool.tile(shape, dt, tag=name or srcd.name,
                               name=name or srcd.name)
                nc.sync.dma_start(out=t[:], in_=srcd[:])
                return t

            iota_t = load_const(iota_d, [P, P])
            ident_t = load_const(ident_d, [P, P])
            dinvw_t = load_const(dinv_w_d, [P, NW])
            dinvl_t = load_const(dinv_l_d, [P, WPD])
            weff_t = load_const(weff, [16, 128])
            wd2_t = load_const(wd2, [128, 256])
            wd3_t = load_const(wd3, [128, 512])
            cts = {nm: load_const(cio[nm], [P, cio[nm].shape[1]]) for nm in cio}
            edst_t = load_const(edst_d, [P, T])
            poolc_t = load_const(pool_d, [P, WPD * 4])
            # bf16 copies of iota/dst for the (bf16) indicator matmuls
            iota_b = cpool.tile([P, P], bf16, tag="iota_b", name="iota_b")
            nc.vector.tensor_copy(iota_b[:], iota_t[:])

            # ---- gather tables in DRAM (t1 fp32: 512B rows = full DMA rate)
            t1dt = bf16 if os.environ.get("KERNEL_T1_BF16", "") == "1" else f32
            t1 = dram.tile([NPAD, 128], t1dt)
            g2_sh = dram.tile([NPD, 256], tdt)
            ccas = "Shared" if os.environ.get("KERNEL_CC_SHARED", "") == "1" else "Local"
            t2 = dram.tile([NPAD, 256], tdt, addr_space=ccas)
            g3_sh = dram.tile([NPD, 512], tdt)
            t3 = dram.tile([NPAD, 512], tdt, addr_space=ccas)

            # ---- phase 1: T1 = dinv * (x @ Weff) for ALL nodes ----
            GB = 8   # windows per t1 write batch
            CHUNK = 16  # node-windows per x_fm chunk DMA
            for cb in range(0, NW, CHUNK):
                cw = min(CHUNK, NW - cb)
                xc = gpool.tile([16, cw * P], f32, tag="xchunk")
                nc.sync.dma_start(out=xc[:], in_=x_fm[:, cb * P : (cb + cw) * P])
                for b0 in range(0, cw, GB):
                    g1b = sbuf.tile([P, GB, 128], t1dt, tag="g1b")
                    for wl in range(GB):
                        wloc = b0 + wl
                        w = cb + wloc
                        ps = psum.tile([P, 128], f32, tag="ps_s", name="ps1")
                        nc.tensor.matmul(
                            ps[:], lhsT=xc[:, wloc * P : (wloc + 1) * P],
                            rhs=weff_t[:], start=True, stop=True)
                        nc.vector.tensor_scalar(
                            out=g1b[:, wl, :], in0=ps[:],
                            scalar1=dinvw_t[:, w : w + 1],
                            scalar2=None, op0=mybir.AluOpType.mult)
                    w0 = cb + b0
                    nc.sync.dma_start(
                        out=t1[w0 * P : (w0 + GB) * P, :].rearrange(
                            "(w p) c -> p w c", p=P),
                        in_=g1b[:])

            tc.strict_bb_all_engine_barrier()

            # zero the gather slots once so never-gathered pad slots are
            # finite (indicator columns kill their contribution)
            tjmax = int(max(tiles_jc[2 * jp, :].sum() + tiles_jc[2 * jp + 1, :].sum()
                            for jp in range(NPAIR)))
            for k in range(2):
                gz = gpool.tile([P, tjmax * 1024], mybir.dt.uint8, tag="gbuf",
                                name=f"gz{k}")
                nc.vector.memset(gz[:], 0)

            # ---- scatter layers ----
            def scatter_layer(tbl, cin, c1t, c0t, lidx):
                """one GCN aggregation layer over this device's 98 windows.
                lidx: 1,2 -> produce G shard for next layer; 3 -> pooling."""
                ldt = t1dt if lidx == 1 else tdt
                io_t = iota_t if ldt == f32 else iota_b
                ed_t = edst_t
                pool_ps = None
                if lidx == 3:
                    pool_ps = [ppool.tile([P, 512], f32, tag=f"pool{q}",
                                          name=f"pool{q}")
                               for q in range(4)]
                for jp in range(NPAIR):
                    j0, j1 = 2 * jp, 2 * jp + 1
                    tj = int(tiles_jc[j0].sum() + tiles_jc[j1].sum())
                    if tj == 0:
                        continue
                    pair_t0 = int(toff_jc[j0, 0])  # first tile of the pair
                    gb = gpool.tile([P, tj, cin], ldt, tag="gbuf")
                    idxw = sbuf.tile([P, tj * 8], mybir.dt.int16, tag="idxw")
                    nc.sync.dma_start(
                        out=idxw[:],
                        in_=eidx_d[:, pair_t0 * 8 : (pair_t0 + tj) * 8])
                    # one gather call per chunk covering both windows
                    for c in range(NCHUNK):
                        t_a, t_b = int(tiles_jc[j0, c]), int(tiles_jc[j1, c])
                        if t_a + t_b == 0:
                            continue
                        tg = int(toff_jc[j0, c]) - pair_t0
                        if t_b > 0:
                            num = int(m_jc[j0, c] + nreal_jc[j1, c])
                        else:
                            num = int(nreal_jc[j0, c])
                        nc.gpsimd.dma_gather(
                            gb[:, tg : tg + t_a + t_b, :],
                            tbl[c * CHROWS : (c + 1) * CHROWS, :],
                            idxw[:, tg * 8 : tg * 8 + ((t_a + t_b) * P) // 16],
                            num, num, cin)
                    # per-window matmul accumulation + post-chain
                    for jw, j in ((0, j0), (1, j1)):
                        ps = psum.tile([P, cin], f32, tag="ps_s")
                        wtiles = []
                        for c in range(NCHUNK):
                            base = int(toff_jc[j, c]) - pair_t0
                            for t in range(int(tiles_jc[j, c])):
                                wtiles.append((base + t, int(toff_jc[j, c]) + t))
                        for k, (tl, tg) in enumerate(wtiles):
                            ind = sbuf.tile([P, P], ldt, tag="ind")
                            nc.vector.tensor_scalar(
                                out=ind[:], in0=io_t[:],
                                scalar1=ed_t[:, tg : tg + 1],
                                scalar2=None, op0=mybir.AluOpType.is_equal)
                            nc.tensor.matmul(
                                ps[:], lhsT=ind[:], rhs=gb[:, tl, :],
                                start=(k == 0), stop=(k == len(wtiles) - 1))
                        t0 = sbuf.tile([P, cin], f32, tag="t0")
                        nc.vector.tensor_scalar(
                            out=t0[:], in0=ps[:], scalar1=dinvl_t[:, j : j + 1],
                            scalar2=None, op0=mybir.AluOpType.mult)
                        cw = c1t.shape[1]
                        xt = sbuf.tile([P, cw], f32, tag="xt")
                        if lidx == 1:  # duplicate 2 conv streams -> 4 bn streams
                            nc.vector.tensor_tensor(
                                out=xt[:, 0:128], in0=t0[:], in1=c1t[:, 0:128],
                                op=mybir.AluOpType.mult)
                            nc.vector.tensor_tensor(
                                out=xt[:, 128:256], in0=t0[:],
                                in1=c1t[:, 128:256], op=mybir.AluOpType.mult)
                        else:
                            nc.vector.tensor_tensor(
                                out=xt[:], in0=t0[:], in1=c1t[:],
                                op=mybir.AluOpType.mult)
                        nc.vector.tensor_tensor(
                            out=xt[:], in0=xt[:], in1=c0t[:],
                            op=mybir.AluOpType.add)
                        nc.scalar.activation(
                            out=xt[:], in_=xt[:],
                            func=mybir.ActivationFunctionType.Relu)

                        if lidx in (1, 2):
                            wd_t = wd2_t if lidx == 1 else wd3_t
                            dout = 64 if lidx == 1 else 128
                            psg = psum.tile([P, 4 * dout], f32, tag="ps_gd",
                                            bufs=1)
                            for h in range(2):  # halves (r1,r2) and (i1,i2)
                                pst = psum.tile([P, P], f32, tag="ps_tr",
                                                bufs=1)
                                nc.tensor.transpose(
                                    out=pst[:],
                                    in_=xt[:, h * 128 : (h + 1) * 128],
                                    identity=ident_t[:])
                                xfm = sbuf.tile([P, P], f32, tag="xfm")
                                nc.vector.tensor_copy(xfm[:], pst[:])
                                for hs in range(2):
                                    s_ = 2 * h + hs
                                    nc.tensor.matmul(
                                        psg[:, s_ * dout : (s_ + 1) * dout],
                                        lhsT=xfm[hs * 64 : (hs + 1) * 64, :],
                                        rhs=wd_t[hs * 64 : (hs + 1) * 64,
                                                 s_ * dout : (s_ + 1) * dout],
                                        start=True, stop=True)
                            gtb = sbuf.tile([P, 2, 4 * dout], tdt, tag="gtb")
                            nc.vector.tensor_scalar(
                                out=gtb[:, jw, :], in0=psg[:],
                                scalar1=dinvl_t[:, j : j + 1],
                                scalar2=None, op0=mybir.AluOpType.mult)
                            if jw == 1:
                                gsh = g2_sh if lidx == 1 else g3_sh
                                nc.sync.dma_start(
                                    out=gsh[j0 * P : (j0 + 2) * P, :].rearrange(
                                        "(w p) c -> p w c", p=P),
                                    in_=gtb[:])
                        else:  # pooling
                            for q in range(4):
                                pind = sbuf.tile([P, P], f32, tag="pind")
                                nc.vector.tensor_scalar(
                                    out=pind[:], in0=iota_t[:],
                                    scalar1=poolc_t[:, j * 4 + q : j * 4 + q + 1],
                                    scalar2=None, op0=mybir.AluOpType.is_equal)
                                nc.tensor.matmul(
                                    pool_ps[q][:], lhsT=pind[:], rhs=xt[:],
                                    start=(j == 0), stop=(j == WPD - 1))
                return pool_ps

            nobarrier = os.environ.get("KERNEL_NO_CC_BARRIER", "") == "1"

            def maybe_barrier():
                if not nobarrier:
                    tc.strict_bb_all_engine_barrier()

            scatter_layer(t1, 128, cts["c1_1"], cts["c0_1"], 1)
            maybe_barrier()
            nc.gpsimd.collective_compute(
                "AllGather", mybir.AluOpType.bypass,
                replica_groups=[list(range(NDEV))],
                ins=[g2_sh.opt()], outs=[t2.opt()])
            maybe_barrier()
            scatter_layer(t2, 256, cts["c1_2"], cts["c0_2"], 2)
            maybe_barrier()
            nc.gpsimd.collective_compute(
                "AllGather", mybir.AluOpType.bypass,
                replica_groups=[list(range(NDEV))],
                ins=[g3_sh.opt()], outs=[t3.opt()])
            maybe_barrier()
            pool_ps = scatter_layer(t3, 512, cts["c1_3"], cts["c0_3"], 3)

            # ---- drain pooled sums ----
            for q in range(4):
                pc = sbuf.tile([P, 512], f32, tag="pcopy")
                nc.vector.tensor_copy(pc[:], pool_ps[q][:])
                nc.sync.dma_start(out=pool_out[q * P : (q + 1) * P, :], in_=pc[:])

    nc.compile()
    _legalize_waits(nc, mybir)
    return nc


# ---------------------------------------------------------------------------
def kernel(x_atom_features, edge_index, batch, params):
    from concourse import bass_utils

    x = np.asarray(x_atom_features, np.float32)
    in_maps, meta = _prep_host(x, np.asarray(edge_index), np.asarray(batch), params)
    nc = _build_program(meta)
    trace = os.environ.get("KERNEL_TRACE", "") == "1"
    res = bass_utils.run_bass_kernel_spmd(
        nc, in_maps, core_ids=list(range(NDEV)), trace=trace)
    global LAST_RESULTS
    LAST_RESULTS = res

    acc = np.zeros((G + GPAD, 512), np.float64)
    for d in range(NDEV):
        b = int(meta["gbase"][d])
        acc[b : b + GPAD, :] += res.results[d]["pool_out"]
    acc = acc[:G] / meta["counts"][:, None]
    acc = acc.astype(np.float32)
    return (acc[:, 0:128], acc[:, 128:256], acc[:, 256:384], acc[:, 384:512])
